# revision 15
# baseline (speedup 1.0000x reference)
"""Compressed Interaction Network (CIN) forward on 8 Trainium2 NeuronCores.

Math (per batch item, m=32 fields, d=64 embed, H=256 hidden):
    x0 = x[i]                          # (m, d)
    h  = x0
    layer l in 0..2:
        z = outer(x0, h) over d        # (m*n, d), z[(a,b),:] = x0[a,:]*h[b,:]
        y = relu(W_l^T z + b_l)        # (H, d)
        xcur, h = split_half(y) (layers 0,1); xcur = h = y (layer 2)
    f = concat(xcur_0, xcur_1, xcur_2) # (512, d)
    out[i] = sum_d(f) @ fc_W + fc_b    # scalar

Mapping: batch 1024 -> 8 cores x 128 items, 16 groups of 8 items per core.
Software-pipelined rounds: round r runs layer 0 of group r-1, layer 1 of
group r-2 and layer 2 of group r-3, so every engine always has a full
round of independent work queued (PE never idles past the HAM window).
 - Layer 0 exploits z's (a,b) symmetry: W0 is host-folded to 528 a<=b
   pairs (5 k-chunks instead of 8); z0 is ONE VectorE op on gathered
   xA/xB operand tensors.
 - z tiles built on VectorE in fp16 (2x mode) from an x broadcast (Bg).
 - Layers 0/1 matmuls in fp16: stationary W chunks [128, 128], moving z
   [128, 512] (8 items x 64 d), fp32 PSUM accumulation over k-chunks.
 - Layer 2 runs entirely as fp8e4 DoubleRow matmuls (contraction 256 per
   instruction, 2x PE rate): z2 is converted fp16->fp8 by ScalarE into a
   field-major carry tile; W2 is host-scaled by 32 into fp8e4's normal
   range and the 1/32 is folded into the eviction scale (ReLU is
   positively homogeneous, biases stay natural). Error attribution showed
   layer-2-only fp8 has ~2.4x the coverage-per-error of layer 1.
 - Bias+ReLU eviction fused on ScalarE; per-item d-sums for the final
   FC via VectorE segmented tensor_reduce into s_tiles. All reduces are
   deferred to a late flush point so the VectorE FIFO never head-of-line
   blocks on evictions gated by the round's last PE block; layer-2's
   z-convert runs a full round ahead of its matmuls (4-deep pipeline,
   double-buffered fp8 carry paid for by shrinking the unused SWDGE
   DMA scratch).
 - Final dot: PE matmul of [128,1] fc weight chunks against [128, 128] sums.
"""

import numpy as np
import ml_dtypes

import concourse.bass as bass
import concourse.tile as tile
from concourse import mybir
from concourse.bass_utils import run_bass_kernel_spmd

N_CORES = 8
B_TOTAL = 1024
B_CORE = B_TOTAL // N_CORES  # 128
M = 32  # num fields
D = 64  # embed dim
H = 256  # conv output channels
GROUP = 8  # items per group (512 moving columns)
N_GROUPS = B_CORE // GROUP  # 16
MD = M * D  # 2048, elements per item row

F16 = mybir.dt.float16
F32 = mybir.dt.float32
F8 = mybir.dt.float8e4
SW = 32.0  # host-side W2 scale (fp8e4 range); undone at eviction
DR = mybir.MatmulPerfMode.DoubleRow
RELU = mybir.ActivationFunctionType.Relu
IDENT = mybir.ActivationFunctionType.Identity
ADD = mybir.AluOpType.add
AXX = mybir.AxisListType.X


def build():
    nc = bass.Bass(dynamic_dma_scratch_size=2048)
    xh = nc.declare_dram_parameter("xh", [B_CORE, M, D], F16, isOutput=False)
    # layer-0 symmetric-pair operands: pair t=128c+p -> (a_t, b_t), a<=b
    # xA[i, p, c, d] = x[i, a_t, d];  xB[i, p, c, d] = x[i, b_t, d]
    xA = nc.declare_dram_parameter("xA", [B_CORE, 128, 5, D], F16, isOutput=False)
    xB = nc.declare_dram_parameter("xB", [B_CORE, 128, 5, D], F16, isOutput=False)
    w0 = nc.declare_dram_parameter("w0", [5, 128, H], F16, isOutput=False)
    w1 = nc.declare_dram_parameter("w1", [32, 128, H], F16, isOutput=False)
    w2 = nc.declare_dram_parameter("w2", [128, 16, 2, H], F8, isOutput=False)
    bia = nc.declare_dram_parameter("bia", [128, 3, 2], F32, isOutput=False)
    fcw = nc.declare_dram_parameter("fcw", [128, 4], F32, isOutput=False)
    fcb = nc.declare_dram_parameter("fcb", [1, 1], F32, isOutput=False)
    out = nc.declare_dram_parameter("out", [B_CORE, 1], F32, isOutput=True)

    with tile.TileContext(nc) as tc:
        with (
            tc.tile_pool(name="consts", bufs=1) as consts,
            tc.tile_pool(name="bpool", bufs=3) as bpool,   # Bg broadcast 32KB
            tc.tile_pool(name="qpool", bufs=2) as qpool,   # xq slices 8KB
            tc.tile_pool(name="rpool", bufs=2) as rpool,   # xr slices 1KB
            tc.tile_pool(name="z0pool", bufs=2) as z0pool, # layer-0 z 8KB
            tc.tile_pool(name="zpool", bufs=4) as zpool,   # layer-1/2 z 4KB
            tc.tile_pool(name="z28p", bufs=2) as z28p,     # l2 fp8 z carry 16KB
            tc.tile_pool(name="hpool", bufs=3) as hpool,   # h tiles 1KB
            tc.tile_pool(name="evpool", bufs=3) as evpool, # relu evictions 1KB
            tc.tile_pool(name="spool", bufs=1) as spool,
            tc.tile_pool(name="ppool", bufs=6, space="PSUM") as ppool,
            tc.tile_pool(name="fcp", bufs=1, space="PSUM") as fcp,
        ):
            # consts are declared up front but their DMAs are interleaved with
            # the first groups' input DMAs below (weights aren't needed until
            # the first matmul; front-loading 4.6MB of them starves round 0).
            w0_sb = consts.tile([128, 5, H], F16, tag="w0")
            w1_sb = consts.tile([128, 32, H], F16, tag="w1")
            w2_sb = consts.tile([128, 16, 2, H], F8, tag="w2")
            bia_sb = consts.tile([128, 3, 2], F32, tag="bia")
            fcw_sb = consts.tile([128, 4], F32, tag="fcw")
            fcb_sb = consts.tile([1, 1], F32, tag="fcb")

            # per-item d-sums of the relu'd xs channels, [channel, item]
            s_tiles = [
                spool.tile([128, B_CORE], F32, tag=f"s{c}", name=f"s{c}")
                for c in range(4)
            ]

            # live tiles per group, carried across rounds
            Bg_t = [None] * N_GROUPS
            Rq_t = [None] * N_GROUPS
            Rg_t = [None] * N_GROUPS
            h1_t = [None] * N_GROUPS
            h2_t = [None] * N_GROUPS

            def dma_group_small(g):
                """xA + xB slices for group g (layer-0 operands, c-major)."""
                i0 = g * GROUP
                Rq = qpool.tile([128, 5, GROUP, D], F16, tag="Rq")
                src = bass.AP(
                    tensor=xA,
                    offset=i0 * 128 * 5 * D,
                    ap=[[5 * D, 128], [D, 5], [128 * 5 * D, GROUP], [1, D]],
                )
                nc.scalar.dma_start(Rq[:], src)
                Rq_t[g] = Rq
                Rg = rpool.tile([128, 5, GROUP, D], F16, tag="Rg")
                src = bass.AP(
                    tensor=xB,
                    offset=i0 * 128 * 5 * D,
                    ap=[[5 * D, 128], [D, 5], [128 * 5 * D, GROUP], [1, D]],
                )
                nc.scalar.dma_start(Rg[:], src)
                Rg_t[g] = Rg

            def dma_group_big(g):
                """x broadcast for layers 1-2: B[p, i, m, d] = x_i[m, d]."""
                i0 = g * GROUP
                Bg = bpool.tile([128, GROUP, M, D], F16, tag="B")
                src = bass.AP(
                    tensor=xh,
                    offset=i0 * MD,
                    ap=[[0, 128], [MD, GROUP], [1, MD]],
                )
                nc.sync.dma_start(Bg[:], src)
                Bg_t[g] = Bg

            def stage0(g):
                """Layer 0 for group g: z0 (one op), 10 MMs, evict, sums."""
                i0 = g * GROUP
                z0 = z0pool.tile([128, 5, GROUP, D], F16, tag="z0")
                nc.vector.tensor_mul(z0[:], Rg_t[g][:], Rq_t[g][:])
                Rq_t[g] = None
                Rg_t[g] = None
                ps = [
                    ppool.tile([128, GROUP * D], F32, tag="yps", name="ps0")
                    for _ in range(2)
                ]
                for q in range(5):
                    for oc in range(2):
                        nc.tensor.matmul(
                            ps[oc][:],
                            w0_sb[:, q, oc * 128 : (oc + 1) * 128],
                            z0[:, q, :, :],
                            start=(q == 0),
                            stop=(q == 4),
                        )
                # chunk1 -> h1 (next layer input); chunk0 -> relu evict + d-sums
                h1 = hpool.tile([128, GROUP, D], F16, tag="h1")
                nc.scalar.activation(h1[:], ps[1][:], RELU, bias=bia_sb[:, 0, 1:2])
                h1_t[g] = h1
                r0 = evpool.tile([128, GROUP, D], F16, tag="ev", bufs=4)
                nc.scalar.activation(r0[:], ps[0][:], RELU, bias=bia_sb[:, 0, 0:1])
                red_q.append((0, i0, r0))


            def stage1(g):
                """Layer 1 (fp16) for group g."""
                i0 = g * GROUP
                ps = [
                    ppool.tile([128, GROUP * D], F32, tag="yps", name="ps")
                    for _ in range(2)
                ]
                for mb in range(8):
                    zt = zpool.tile([128, GROUP, 4, D], F16, tag="z")
                    nc.vector.tensor_mul(
                        zt[:],
                        h1_t[g][:, :, None, :].to_broadcast((128, GROUP, 4, D)),
                        Bg_t[g][:, :, 4 * mb : 4 * mb + 4, :],
                    )
                    for mm in range(4):
                        m = 4 * mb + mm
                        for oc in range(2):
                            nc.tensor.matmul(
                                ps[oc][:],
                                w1_sb[:, m, oc * 128 : (oc + 1) * 128],
                                zt[:, :, mm, :],
                                start=(m == 0),
                                stop=(m == 31),
                            )
                h2 = hpool.tile([128, GROUP, D], F16, tag="h2")
                nc.scalar.activation(h2[:], ps[1][:], RELU, bias=bia_sb[:, 1, 1:2])
                h2_t[g] = h2
                h1_t[g] = None
                r1 = evpool.tile([128, GROUP, D], F16, tag="ev", bufs=4)
                nc.scalar.activation(r1[:], ps[0][:], RELU, bias=bia_sb[:, 1, 0:1])
                red_q.append((1, i0, r1))

            z28_t = [None] * N_GROUPS
            red_q = []

            def flush_reduces():
                for c, i0, r in red_q:
                    nc.vector.tensor_reduce(
                        s_tiles[c][:, i0 : i0 + GROUP], r[:], AXX, ADD
                    )
                red_q.clear()

            def stage2a(g):
                """Layer 2 z build + fp8 convert (one round ahead of the MMs)."""
                z28 = z28p.tile([128, M, GROUP, D], F8, tag="z28")
                for mb in range(8):
                    zt = zpool.tile([128, 4, GROUP, D], F16, tag="z", name="z2f")
                    nc.vector.tensor_mul(
                        zt[:],
                        h2_t[g][:, None, :, :].to_broadcast((128, 4, GROUP, D)),
                        Bg_t[g][:, :, 4 * mb : 4 * mb + 4, :].rearrange(
                            "p i a d -> p a i d"
                        ),
                    )
                    nc.scalar.activation(
                        z28[:, 4 * mb : 4 * mb + 4, :, :], zt[:], IDENT
                    )
                z28_t[g] = z28
                h2_t[g] = None
                Bg_t[g] = None

            def stage2b(g):
                """Layer 2 fp8-DoubleRow matmuls + evictions + d-sums."""
                z28 = z28_t[g]
                z28_t[g] = None
                i0 = g * GROUP
                ps = [
                    ppool.tile([128, GROUP * D], F32, tag="yps", name="ps")
                    for _ in range(2)
                ]
                for t in range(16):
                    for oc in range(2):
                        nc.tensor.matmul(
                            ps[oc][:],
                            w2_sb[:, t, :, oc * 128 : (oc + 1) * 128],
                            z28[:, 2 * t : 2 * t + 2, :, :],
                            start=(t == 0),
                            stop=(t == 15),
                            perf_mode=DR,
                        )
                for oc in range(2):
                    r2 = evpool.tile(
                        [128, GROUP, D], F16, tag="ev2", bufs=4, name="r2"
                    )
                    nc.scalar.activation(
                        r2[:],
                        ps[oc][:],
                        RELU,
                        bias=bia_sb[:, 2, oc : oc + 1],
                        scale=1.0 / SW,
                    )
                    red_q.append((2 + oc, i0, r2))

            # pipeline-fill DMA order: group 0 inputs first, then weights
            dma_group_small(0)
            nc.sync.dma_start(w0_sb[:], w0[:].rearrange("c k o -> k c o"))
            nc.sync.dma_start(bia_sb[:], bia[:])
            dma_group_small(1)
            dma_group_big(0)
            nc.sync.dma_start(w1_sb[:], w1[:].rearrange("c k o -> k c o"))
            nc.sync.dma_start(w2_sb[:], w2[:])
            nc.sync.dma_start(fcw_sb[:], fcw[:])
            nc.sync.dma_start(fcb_sb[:], fcb[:])

            # --- software-pipelined rounds ---
            for r in range(N_GROUPS + 5):
                if 2 <= r < N_GROUPS:
                    dma_group_small(r)
                if 1 <= r - 1 < N_GROUPS:
                    dma_group_big(r - 1)
                if 0 <= r - 1 < N_GROUPS:
                    stage0(r - 1)
                if 0 <= r - 2 < N_GROUPS:
                    stage1(r - 2)
                if 0 <= r - 3 < N_GROUPS:
                    stage2a(r - 3)
                flush_reduces()
                if 0 <= r - 4 < N_GROUPS:
                    stage2b(r - 4)

            # ---------- final FC: out[i] = sum_c fcw[c] * s[c, i] + fcb ----------
            fc_ps = fcp.tile([1, B_CORE], F32, tag="fc")
            for c in range(4):
                nc.tensor.matmul(
                    fc_ps[:],
                    fcw_sb[:, c : c + 1],
                    s_tiles[c][:],
                    start=(c == 0),
                    stop=(c == 3),
                )
            osb = consts.tile([1, B_CORE], F32, tag="osb")
            nc.scalar.activation(osb[:], fc_ps[:], IDENT, bias=fcb_sb[0:1, 0:1])
            nc.sync.dma_start(out[:], osb[:])

    _legalize_waits(nc)
    return nc


def _legalize_waits(nc, max_waits=1):
    """walrus codegen allows at most 2 semaphore waits per instruction; spill
    the excess onto NoOps injected just before the offender on the same
    engine (same-engine FIFO makes this ordering-equivalent)."""
    for bb in nc.main_func.blocks:
        insts = bb.instructions
        new_list = []
        changed = False
        for ins in insts:
            si = ins.sync_info
            if si is not None and si.on_wait and len(si.on_wait) > max_waits:
                waits = list(si.on_wait)
                extra, keep = waits[:-max_waits], waits[-max_waits:]
                k = 0
                while k < len(extra):
                    chunk = extra[k : k + max_waits]
                    nop = mybir.InstNoOp(name=f"{ins.name}-w{k}", ins=[], outs=[])
                    nop.engine = ins.engine
                    nop.sync_info = mybir.SyncInfo(on_wait=chunk, on_update=[])
                    new_list.append(nop)
                    k += max_waits
                ins.sync_info = mybir.SyncInfo(
                    on_wait=keep,
                    on_update=list(si.on_update) if si.on_update else [],
                )
                changed = True
            new_list.append(ins)
        if changed:
            if hasattr(bb, "set_instructions"):
                bb.set_instructions(new_list)
            else:
                insts.clear()
                insts.extend(new_list)
                if len(bb.instructions) != len(new_list):
                    bb.instructions = new_list


def prep_inputs(x, W0, b0, W1, b1, W2, b2, fc_W, fc_b):
    """Host-side reshape/cast into the per-core input maps."""
    xh = np.ascontiguousarray(x.astype(np.float16))
    # symmetric-fold layer 0: 528 (a<=b) pairs, padded to 5 chunks of 128
    ab = np.array([(a, b) for a in range(M) for b in range(a, M)])  # [528, 2]
    ab = np.concatenate([ab, np.zeros((640 - len(ab), 2), np.int64)])
    xq = np.ascontiguousarray(
        xh[:, ab[:, 0], :].reshape(B_TOTAL, 5, 128, D).transpose(0, 2, 1, 3)
    )  # xA[i, p, c, d]
    xr = np.ascontiguousarray(
        xh[:, ab[:, 1], :].reshape(B_TOTAL, 5, 128, D).transpose(0, 2, 1, 3)
    )  # xB[i, p, c, d]
    W0v = W0.reshape(M, M, H)
    W0s = W0v[ab[:, 0], ab[:, 1]] + np.where(
        (ab[:, 0] != ab[:, 1])[:, None], W0v[ab[:, 1], ab[:, 0]], 0.0
    )
    W0s[528:] = 0.0
    w0 = np.ascontiguousarray(W0s.astype(np.float16).reshape(5, 128, H))
    w1 = np.ascontiguousarray(W1.astype(np.float16).reshape(32, 128, H))
    p_ = np.arange(128)
    t_, j_ = np.meshgrid(np.arange(16), np.arange(2), indexing="ij")
    k2 = (2 * t_[None] + j_[None]) * 128 + p_[:, None, None]
    w2 = np.ascontiguousarray((SW * W2[k2]).astype(ml_dtypes.float8_e4m3))
    bia = np.ascontiguousarray(
        np.stack([b0, b1, b2]).reshape(3, 2, 128).transpose(2, 0, 1).astype(np.float32)
    )
    fcw = np.ascontiguousarray(fc_W.reshape(4, 128).T.astype(np.float32))
    fcb = np.ascontiguousarray(fc_b.reshape(1, 1).astype(np.float32))
    shared = {"bia": bia, "fcw": fcw, "fcb": fcb, "w0": w0, "w1": w1, "w2": w2}
    return [
        {
            "xh": xh[i * B_CORE : (i + 1) * B_CORE],
            "xB": xr[i * B_CORE : (i + 1) * B_CORE],
            "xA": xq[i * B_CORE : (i + 1) * B_CORE],
            **shared,
        }
        for i in range(N_CORES)
    ]


_NC = None


def _get_nc():
    global _NC
    if _NC is None:
        _NC = build()
    return _NC


def kernel(**inputs):
    in_maps = prep_inputs(**inputs)
    res = run_bass_kernel_spmd(_get_nc(), in_maps, list(range(N_CORES)))
    return np.ascontiguousarray(
        np.concatenate([r["out"] for r in res.results], axis=0).astype(np.float32)
    )


# revision 16
# speedup vs baseline: 1.1458x; 1.1458x over previous
"""Compressed Interaction Network (CIN) forward on 8 Trainium2 NeuronCores.

Math (per batch item, m=32 fields, d=64 embed, H=256 hidden):
    x0 = x[i]                          # (m, d)
    h  = x0
    layer l in 0..2:
        z = outer(x0, h) over d        # (m*n, d), z[(a,b),:] = x0[a,:]*h[b,:]
        y = relu(W_l^T z + b_l)        # (H, d)
        xcur, h = split_half(y) (layers 0,1); xcur = h = y (layer 2)
    f = concat(xcur_0, xcur_1, xcur_2) # (512, d)
    out[i] = sum_d(f) @ fc_W + fc_b    # scalar

Mapping: batch 1024 -> 8 cores x 128 items, 16 groups of 8 items per core.
Software-pipelined rounds: round r runs layer 0 of group r-1, layer 1 of
group r-2 and layer 2 of group r-3, so every engine always has a full
round of independent work queued (PE never idles past the HAM window).
 - Layer 0 exploits z's (a,b) symmetry: W0 is host-folded to 528 a<=b
   pairs (5 k-chunks instead of 8); z0 is ONE VectorE op on gathered
   xA/xB operand tensors.
 - z tiles built on VectorE in fp16 (2x mode) from an x broadcast (Bg).
 - Layers 0/1 matmuls in fp16: stationary W chunks [128, 128], moving z
   [128, 512] (8 items x 64 d), fp32 PSUM accumulation over k-chunks.
 - Layer 2 runs entirely as fp8e4 DoubleRow matmuls (contraction 256 per
   instruction, 2x PE rate): z2 is converted fp16->fp8 by ScalarE into a
   field-major carry tile; W2 is host-scaled by 32 into fp8e4's normal
   range and the 1/32 is folded into the eviction scale (ReLU is
   positively homogeneous, biases stay natural). Error attribution showed
   layer-2-only fp8 has ~2.4x the coverage-per-error of layer 1.
 - Bias+ReLU eviction fused on ScalarE; per-item d-sums for the final
   FC via VectorE segmented tensor_reduce into s_tiles. All reduces are
   deferred to a late flush point so the VectorE FIFO never head-of-line
   blocks on evictions gated by the round's last PE block; layer-2's
   z-convert runs a full round ahead of its matmuls (4-deep pipeline,
   double-buffered fp8 carry paid for by shrinking the unused SWDGE
   DMA scratch).
 - Final dot: PE matmul of [128,1] fc weight chunks against [128, 128] sums.
"""

import numpy as np
import ml_dtypes

import concourse.bass as bass
import concourse.tile as tile
from concourse import mybir
from concourse.bass_utils import run_bass_kernel_spmd

N_CORES = 8
B_TOTAL = 1024
B_CORE = B_TOTAL // N_CORES  # 128
M = 32  # num fields
D = 64  # embed dim
H = 256  # conv output channels
GROUP = 8  # items per group (512 moving columns)
N_GROUPS = B_CORE // GROUP  # 16
MD = M * D  # 2048, elements per item row

F16 = mybir.dt.float16
F32 = mybir.dt.float32
F8 = mybir.dt.float8e4
SW = 32.0  # host-side W2 scale (fp8e4 range); undone at eviction
DR = mybir.MatmulPerfMode.DoubleRow
RELU = mybir.ActivationFunctionType.Relu
IDENT = mybir.ActivationFunctionType.Identity
ADD = mybir.AluOpType.add
AXX = mybir.AxisListType.X


def build():
    nc = bass.Bass(dynamic_dma_scratch_size=2048)
    xh = nc.declare_dram_parameter("xh", [B_CORE, M, D], F16, isOutput=False)
    # layer-0 symmetric-pair operands: pair t=128c+p -> (a_t, b_t), a<=b
    # xA[i, p, c, d] = x[i, a_t, d];  xB[i, p, c, d] = x[i, b_t, d]
    xA = nc.declare_dram_parameter("xA", [B_CORE, 128, 5, D], F16, isOutput=False)
    xB = nc.declare_dram_parameter("xB", [B_CORE, 128, 5, D], F16, isOutput=False)
    w0 = nc.declare_dram_parameter("w0", [5, 128, H], F16, isOutput=False)
    w1 = nc.declare_dram_parameter("w1", [32, 128, H], F16, isOutput=False)
    w2 = nc.declare_dram_parameter("w2", [128, 16, 2, H], F8, isOutput=False)
    bia = nc.declare_dram_parameter("bia", [128, 3, 2], F32, isOutput=False)
    fcw = nc.declare_dram_parameter("fcw", [128, 4], F32, isOutput=False)
    fcb = nc.declare_dram_parameter("fcb", [1, 1], F32, isOutput=False)
    out = nc.declare_dram_parameter("out", [B_CORE, 1], F32, isOutput=True)

    with tile.TileContext(nc) as tc:
        with (
            tc.tile_pool(name="consts", bufs=1) as consts,
            tc.tile_pool(name="bpool", bufs=3) as bpool,   # Bg broadcast 32KB
            tc.tile_pool(name="qpool", bufs=2) as qpool,   # xq slices 8KB
            tc.tile_pool(name="rpool", bufs=2) as rpool,   # xr slices 1KB
            tc.tile_pool(name="z0pool", bufs=2) as z0pool, # layer-0 z 8KB
            tc.tile_pool(name="zpool", bufs=4) as zpool,   # layer-1/2 z 4KB
            tc.tile_pool(name="z28p", bufs=2) as z28p,     # l2 fp8 z carry 16KB
            tc.tile_pool(name="hpool", bufs=3) as hpool,   # h tiles 1KB
            tc.tile_pool(name="evpool", bufs=3) as evpool, # relu evictions 1KB
            tc.tile_pool(name="spool", bufs=1) as spool,
            tc.tile_pool(name="ppool", bufs=6, space="PSUM") as ppool,
            tc.tile_pool(name="fcp", bufs=1, space="PSUM") as fcp,
        ):
            # consts are declared up front but their DMAs are interleaved with
            # the first groups' input DMAs below (weights aren't needed until
            # the first matmul; front-loading 4.6MB of them starves round 0).
            w0_sb = consts.tile([128, 5, H], F16, tag="w0")
            w1_sb = consts.tile([128, 32, H], F16, tag="w1")
            w2_sb = consts.tile([128, 16, 2, H], F8, tag="w2")
            bia_sb = consts.tile([128, 3, 2], F32, tag="bia")
            fcw_sb = consts.tile([128, 4], F32, tag="fcw")
            fcb_sb = consts.tile([1, 1], F32, tag="fcb")

            # per-item d-sums of the relu'd xs channels, [channel, item]
            s_tiles = [
                spool.tile([128, B_CORE], F32, tag=f"s{c}", name=f"s{c}")
                for c in range(4)
            ]

            # live tiles per group, carried across rounds
            Bg_t = [None] * N_GROUPS
            Rq_t = [None] * N_GROUPS
            Rg_t = [None] * N_GROUPS
            h1_t = [None] * N_GROUPS
            h2_t = [None] * N_GROUPS

            def dma_group_small(g):
                """xA + xB slices for group g (layer-0 operands, c-major)."""
                i0 = g * GROUP
                Rq = qpool.tile([128, 5, GROUP, D], F16, tag="Rq")
                src = bass.AP(
                    tensor=xA,
                    offset=i0 * 128 * 5 * D,
                    ap=[[5 * D, 128], [D, 5], [128 * 5 * D, GROUP], [1, D]],
                )
                nc.sync.dma_start(Rq[:], src)
                Rq_t[g] = Rq
                Rg = rpool.tile([128, 5, GROUP, D], F16, tag="Rg")
                src = bass.AP(
                    tensor=xB,
                    offset=i0 * 128 * 5 * D,
                    ap=[[5 * D, 128], [D, 5], [128 * 5 * D, GROUP], [1, D]],
                )
                nc.sync.dma_start(Rg[:], src)
                Rg_t[g] = Rg

            def dma_group_big(g):
                """x broadcast for layers 1-2: B[p, i, m, d] = x_i[m, d]."""
                i0 = g * GROUP
                Bg = bpool.tile([128, GROUP, M, D], F16, tag="B")
                src = bass.AP(
                    tensor=xh,
                    offset=i0 * MD,
                    ap=[[0, 128], [MD, GROUP], [1, MD]],
                )
                nc.sync.dma_start(Bg[:], src)
                Bg_t[g] = Bg

            def stage0(g):
                """Layer 0 for group g: z0 (one op), 10 MMs, evict, sums."""
                i0 = g * GROUP
                z0 = z0pool.tile([128, 5, GROUP, D], F16, tag="z0")
                nc.vector.tensor_mul(z0[:], Rg_t[g][:], Rq_t[g][:])
                Rq_t[g] = None
                Rg_t[g] = None
                ps = [
                    ppool.tile([128, GROUP * D], F32, tag="yps", name="ps0")
                    for _ in range(2)
                ]
                for q in range(5):
                    for oc in range(2):
                        nc.tensor.matmul(
                            ps[oc][:],
                            w0_sb[:, q, oc * 128 : (oc + 1) * 128],
                            z0[:, q, :, :],
                            start=(q == 0),
                            stop=(q == 4),
                        )
                # chunk1 -> h1 (next layer input); chunk0 -> relu evict + d-sums
                h1 = hpool.tile([128, GROUP, D], F16, tag="h1")
                nc.scalar.activation(h1[:], ps[1][:], RELU, bias=bia_sb[:, 0, 1:2])
                h1_t[g] = h1
                r0 = evpool.tile([128, GROUP, D], F16, tag="ev", bufs=4)
                nc.scalar.activation(r0[:], ps[0][:], RELU, bias=bia_sb[:, 0, 0:1])
                red_q.append((0, i0, r0))


            def stage1(g):
                """Layer 1 (fp16) for group g."""
                i0 = g * GROUP
                ps = [
                    ppool.tile([128, GROUP * D], F32, tag="yps", name="ps")
                    for _ in range(2)
                ]
                for mb in range(8):
                    zt = zpool.tile([128, GROUP, 4, D], F16, tag="z")
                    nc.vector.tensor_mul(
                        zt[:],
                        h1_t[g][:, :, None, :].to_broadcast((128, GROUP, 4, D)),
                        Bg_t[g][:, :, 4 * mb : 4 * mb + 4, :],
                    )
                    for mm in range(4):
                        m = 4 * mb + mm
                        for oc in range(2):
                            nc.tensor.matmul(
                                ps[oc][:],
                                w1_sb[:, m, oc * 128 : (oc + 1) * 128],
                                zt[:, :, mm, :],
                                start=(m == 0),
                                stop=(m == 31),
                            )
                h2 = hpool.tile([128, GROUP, D], F16, tag="h2")
                nc.scalar.activation(h2[:], ps[1][:], RELU, bias=bia_sb[:, 1, 1:2])
                h2_t[g] = h2
                h1_t[g] = None
                r1 = evpool.tile([128, GROUP, D], F16, tag="ev", bufs=4)
                nc.scalar.activation(r1[:], ps[0][:], RELU, bias=bia_sb[:, 1, 0:1])
                red_q.append((1, i0, r1))

            z28_t = [None] * N_GROUPS
            red_q = []

            def flush_reduces():
                for c, i0, r in red_q:
                    nc.vector.tensor_reduce(
                        s_tiles[c][:, i0 : i0 + GROUP], r[:], AXX, ADD
                    )
                red_q.clear()

            def stage2a(g):
                """Layer 2 z build + fp8 convert (one round ahead of the MMs)."""
                z28 = z28p.tile([128, M, GROUP, D], F8, tag="z28")
                for mb in range(8):
                    zt = zpool.tile([128, 4, GROUP, D], F16, tag="z", name="z2f")
                    nc.vector.tensor_mul(
                        zt[:],
                        h2_t[g][:, None, :, :].to_broadcast((128, 4, GROUP, D)),
                        Bg_t[g][:, :, 4 * mb : 4 * mb + 4, :].rearrange(
                            "p i a d -> p a i d"
                        ),
                    )
                    nc.scalar.activation(
                        z28[:, 4 * mb : 4 * mb + 4, :, :], zt[:], IDENT
                    )
                z28_t[g] = z28
                h2_t[g] = None
                Bg_t[g] = None

            def stage2b(g):
                """Layer 2 fp8-DoubleRow matmuls + evictions + d-sums."""
                z28 = z28_t[g]
                z28_t[g] = None
                i0 = g * GROUP
                ps = [
                    ppool.tile([128, GROUP * D], F32, tag="yps", name="ps")
                    for _ in range(2)
                ]
                for t in range(16):
                    for oc in range(2):
                        nc.tensor.matmul(
                            ps[oc][:],
                            w2_sb[:, t, :, oc * 128 : (oc + 1) * 128],
                            z28[:, 2 * t : 2 * t + 2, :, :],
                            start=(t == 0),
                            stop=(t == 15),
                            perf_mode=DR,
                        )
                for oc in range(2):
                    r2 = evpool.tile(
                        [128, GROUP, D], F16, tag="ev2", bufs=4, name="r2"
                    )
                    nc.scalar.activation(
                        r2[:],
                        ps[oc][:],
                        RELU,
                        bias=bia_sb[:, 2, oc : oc + 1],
                        scale=1.0 / SW,
                    )
                    red_q.append((2 + oc, i0, r2))

            # pipeline-fill DMA order: group 0 inputs first, then weights
            dma_group_small(0)
            nc.sync.dma_start(w0_sb[:], w0[:].rearrange("c k o -> k c o"))
            nc.sync.dma_start(bia_sb[:], bia[:])
            dma_group_small(1)
            dma_group_big(0)
            nc.sync.dma_start(w1_sb[:], w1[:].rearrange("c k o -> k c o"))
            nc.sync.dma_start(w2_sb[:], w2[:])
            nc.sync.dma_start(fcw_sb[:], fcw[:])
            nc.sync.dma_start(fcb_sb[:], fcb[:])

            # --- software-pipelined rounds ---
            for r in range(N_GROUPS + 5):
                if 2 <= r < N_GROUPS:
                    dma_group_small(r)
                if 1 <= r - 1 < N_GROUPS:
                    dma_group_big(r - 1)
                if 0 <= r - 1 < N_GROUPS:
                    stage0(r - 1)
                if 0 <= r - 2 < N_GROUPS:
                    stage1(r - 2)
                if 0 <= r - 3 < N_GROUPS:
                    stage2a(r - 3)
                flush_reduces()
                if 0 <= r - 4 < N_GROUPS:
                    stage2b(r - 4)

            # ---------- final FC: out[i] = sum_c fcw[c] * s[c, i] + fcb ----------
            fc_ps = fcp.tile([1, B_CORE], F32, tag="fc")
            for c in range(4):
                nc.tensor.matmul(
                    fc_ps[:],
                    fcw_sb[:, c : c + 1],
                    s_tiles[c][:],
                    start=(c == 0),
                    stop=(c == 3),
                )
            osb = consts.tile([1, B_CORE], F32, tag="osb")
            nc.scalar.activation(osb[:], fc_ps[:], IDENT, bias=fcb_sb[0:1, 0:1])
            nc.sync.dma_start(out[:], osb[:])

    _legalize_waits(nc)
    return nc


def _legalize_waits(nc, max_waits=1):
    """walrus codegen allows at most 2 semaphore waits per instruction; spill
    the excess onto NoOps injected just before the offender on the same
    engine (same-engine FIFO makes this ordering-equivalent)."""
    for bb in nc.main_func.blocks:
        insts = bb.instructions
        new_list = []
        changed = False
        for ins in insts:
            si = ins.sync_info
            if si is not None and si.on_wait and len(si.on_wait) > max_waits:
                waits = list(si.on_wait)
                extra, keep = waits[:-max_waits], waits[-max_waits:]
                k = 0
                while k < len(extra):
                    chunk = extra[k : k + max_waits]
                    nop = mybir.InstNoOp(name=f"{ins.name}-w{k}", ins=[], outs=[])
                    nop.engine = ins.engine
                    nop.sync_info = mybir.SyncInfo(on_wait=chunk, on_update=[])
                    new_list.append(nop)
                    k += max_waits
                ins.sync_info = mybir.SyncInfo(
                    on_wait=keep,
                    on_update=list(si.on_update) if si.on_update else [],
                )
                changed = True
            new_list.append(ins)
        if changed:
            if hasattr(bb, "set_instructions"):
                bb.set_instructions(new_list)
            else:
                insts.clear()
                insts.extend(new_list)
                if len(bb.instructions) != len(new_list):
                    bb.instructions = new_list


def prep_inputs(x, W0, b0, W1, b1, W2, b2, fc_W, fc_b):
    """Host-side reshape/cast into the per-core input maps."""
    xh = np.ascontiguousarray(x.astype(np.float16))
    # symmetric-fold layer 0: 528 (a<=b) pairs, padded to 5 chunks of 128
    ab = np.array([(a, b) for a in range(M) for b in range(a, M)])  # [528, 2]
    ab = np.concatenate([ab, np.zeros((640 - len(ab), 2), np.int64)])
    xq = np.ascontiguousarray(
        xh[:, ab[:, 0], :].reshape(B_TOTAL, 5, 128, D).transpose(0, 2, 1, 3)
    )  # xA[i, p, c, d]
    xr = np.ascontiguousarray(
        xh[:, ab[:, 1], :].reshape(B_TOTAL, 5, 128, D).transpose(0, 2, 1, 3)
    )  # xB[i, p, c, d]
    W0v = W0.reshape(M, M, H)
    W0s = W0v[ab[:, 0], ab[:, 1]] + np.where(
        (ab[:, 0] != ab[:, 1])[:, None], W0v[ab[:, 1], ab[:, 0]], 0.0
    )
    W0s[528:] = 0.0
    w0 = np.ascontiguousarray(W0s.astype(np.float16).reshape(5, 128, H))
    w1 = np.ascontiguousarray(W1.astype(np.float16).reshape(32, 128, H))
    p_ = np.arange(128)
    t_, j_ = np.meshgrid(np.arange(16), np.arange(2), indexing="ij")
    k2 = (2 * t_[None] + j_[None]) * 128 + p_[:, None, None]
    w2 = np.ascontiguousarray((SW * W2[k2]).astype(ml_dtypes.float8_e4m3))
    bia = np.ascontiguousarray(
        np.stack([b0, b1, b2]).reshape(3, 2, 128).transpose(2, 0, 1).astype(np.float32)
    )
    fcw = np.ascontiguousarray(fc_W.reshape(4, 128).T.astype(np.float32))
    fcb = np.ascontiguousarray(fc_b.reshape(1, 1).astype(np.float32))
    shared = {"bia": bia, "fcw": fcw, "fcb": fcb, "w0": w0, "w1": w1, "w2": w2}
    return [
        {
            "xh": xh[i * B_CORE : (i + 1) * B_CORE],
            "xB": xr[i * B_CORE : (i + 1) * B_CORE],
            "xA": xq[i * B_CORE : (i + 1) * B_CORE],
            **shared,
        }
        for i in range(N_CORES)
    ]


_NC = None


def _get_nc():
    global _NC
    if _NC is None:
        _NC = build()
    return _NC


def kernel(**inputs):
    in_maps = prep_inputs(**inputs)
    res = run_bass_kernel_spmd(_get_nc(), in_maps, list(range(N_CORES)))
    return np.ascontiguousarray(
        np.concatenate([r["out"] for r in res.results], axis=0).astype(np.float32)
    )
